# revision 6
# baseline (speedup 1.0000x reference)
"""Trainium2 Bass kernel: 2D valid cross-correlation (3x3) + bias on 8192x8192 fp32.

Strategy:
  - Row-shard X across 8 NeuronCores with a 2-row halo handled by host-side
    overlapped slicing (each core gets a 1026x8192 slab; core 7's slab is
    shifted up by 2 rows so all cores run an identical SPMD program).
  - On each core: conv2d(3x3) = 3 PSUM-accumulating matmuls per output tile
    with banded "shift" matrices built from the 3x3 weight. The banded matrix
    B_dj[p, o] = w[p - o, dj] turns the partition-axis (row) shifts into a
    matmul contraction; the column shifts dj are free-axis offsets of the rhs.
    float32r inputs run the PE at ~2 cycles/row (vs 4 for fp32) at ~tf32
    precision.
  - Loads/stores are split into ~1MB column chunks on separate HWDGE rings
    (loads on sync, stores on scalar) so compute starts ~5us in and the
    DMA engines stay saturated; weights/bias ride SWDGE (gpsimd).
  - Matmuls are grouped dj-outer over 4 PSUM banks (4 col-tiles share one
    LDWEIGHTS target) to keep PE bursts dense; DVE evacuates PSUM -> SBUF
    fused with the bias add.
"""

import os
import sys

import numpy as np

for _p in (
    "/opt/trn_rl_repo",
    "/root/.axon_site/_ro/trn_rl_repo",
    "/root/.axon_site/_ro/pypackages",
    "/opt/pypackages",
):
    if os.path.isdir(_p) and _p not in sys.path:
        sys.path.append(_p)

import concourse.bacc as bacc
import concourse.mybir as mybir
import concourse.tile as tile
from concourse.bass_utils import run_bass_kernel_spmd

N_CORES = 8
H = W = 8192
KH = KW = 3
OH = H - KH + 1  # 8190
OW = W - KW + 1  # 8190
ROWS_PER_CORE = 1024  # output rows produced per core (core 7: first 2 dropped)
SLAB_IN_ROWS = 1026  # input rows per core slab
SLAB_IN = 128  # input rows per row-slab tile
SLAB_OUT = 126  # output rows per row-slab tile
N_FULL_SLABS = 8  # 8 * 126 = 1008
TAIL_IN = SLAB_IN_ROWS - N_FULL_SLABS * SLAB_OUT  # 18
TAIL_OUT = ROWS_PER_CORE - N_FULL_SLABS * SLAB_OUT  # 16
COL_TILE = 512
N_COLT = (OW + COL_TILE - 1) // COL_TILE  # 16 (15*512 + 510)
GROUP = 4  # col-tiles per dj-outer matmul group (= PSUM banks per group)
N_GROUPS = N_COLT // GROUP  # 4

# Column chunking for DMA: ~1MB chunks so transfers stay efficient while the
# pipeline gets fine-grained dependencies (compute starts after chunk 0).
IN_CHUNKS = [(0, 2050), (2050, 4098), (4098, 6146), (6146, 8192)]
OUT_CHUNKS = [(0, 2048), (2048, 4096), (4096, 6144), (6144, 8190)]

_NC = None
LAST_RESULTS = None


def _build():
    nc = bacc.Bacc(
        "TRN2", target_bir_lowering=False, debug=False, num_devices=N_CORES
    )
    # float32r: same bits as fp32, but marks the data as feedable to the PE's
    # fp32r matmul mode (the BIR verifier requires fp32r matmul inputs to be
    # *produced* as fp32r, so the dtype is set at the DRAM/DMA level).
    xs = nc.dram_tensor(
        "xs", [SLAB_IN_ROWS, W], mybir.dt.float32r, kind="ExternalInput"
    )
    bands = nc.dram_tensor(
        "bands", [SLAB_IN, KW, SLAB_OUT], mybir.dt.float32r, kind="ExternalInput"
    )
    biasb = nc.dram_tensor("biasb", [SLAB_IN, 1], mybir.dt.float32, kind="ExternalInput")
    out = nc.dram_tensor(
        "out", [ROWS_PER_CORE, OW], mybir.dt.float32, kind="ExternalOutput"
    )

    f32r = mybir.dt.float32r
    f32 = mybir.dt.float32

    with tile.TileContext(nc) as tc:
        with (
            tc.tile_pool(name="const", bufs=1) as cpool,
            tc.tile_pool(name="inp", bufs=3) as ipool,
            tc.tile_pool(name="outp", bufs=2) as opool,
            tc.tile_pool(name="psum", bufs=2 * GROUP, space="PSUM") as pspool,
        ):
            bt = cpool.tile([SLAB_IN, KW, SLAB_OUT], f32r, tag="bt")
            nc.gpsimd.dma_start(bt[:], bands.ap())
            bias_t = cpool.tile([SLAB_IN, 1], f32, tag="bias")
            nc.gpsimd.dma_start(bias_t[:], biasb.ap())

            for s in range(N_FULL_SLABS + 1):
                in_rows = SLAB_IN if s < N_FULL_SLABS else TAIL_IN
                out_rows = SLAB_OUT if s < N_FULL_SLABS else TAIL_OUT
                r0 = s * SLAB_OUT

                it = ipool.tile([SLAB_IN, W], f32r, tag="it", name=f"it{s}")
                for a, b in IN_CHUNKS:
                    nc.sync.dma_start(
                        it[:in_rows, a:b], xs.ap()[r0 : r0 + in_rows, a:b]
                    )

                ot = opool.tile([SLAB_OUT, OW], f32, tag="ot", name=f"ot{s}")

                for g in range(N_GROUPS):
                    pss = [
                        pspool.tile(
                            [SLAB_OUT, COL_TILE], f32, tag="ps", name=f"ps{s}_{g}_{t}"
                        )
                        for t in range(GROUP)
                    ]
                    for dj in range(KW):
                        for t in range(GROUP):
                            j = GROUP * g + t
                            c0 = j * COL_TILE
                            n = min(COL_TILE, OW - c0)
                            nc.tensor.matmul(
                                pss[t][:out_rows, :n],
                                bt[:in_rows, dj, :out_rows],
                                it[:in_rows, c0 + dj : c0 + dj + n],
                                start=(dj == 0),
                                stop=(dj == KW - 1),
                            )
                    for t in range(GROUP):
                        j = GROUP * g + t
                        c0 = j * COL_TILE
                        n = min(COL_TILE, OW - c0)
                        nc.vector.tensor_scalar_add(
                            ot[:out_rows, c0 : c0 + n],
                            pss[t][:out_rows, :n],
                            bias_t[:out_rows, :],
                        )
                    a, b = OUT_CHUNKS[g]
                    nc.scalar.dma_start(
                        out.ap()[r0 : r0 + out_rows, a:b], ot[:out_rows, a:b]
                    )

    nc.compile()
    return nc


def kernel(X, weight, bias):
    global _NC, LAST_RESULTS
    X = np.ascontiguousarray(np.asarray(X, dtype=np.float32))
    weight = np.asarray(weight, dtype=np.float32)
    bias = np.asarray(bias, dtype=np.float32).reshape(-1)

    if _NC is None:
        _NC = _build()
    nc = _NC

    # Banded shift matrices: bands[p, dj, o] = w[p - o, dj] for 0 <= p-o < 3.
    bands = np.zeros((SLAB_IN, KW, SLAB_OUT), dtype=np.float32)
    o = np.arange(SLAB_OUT)
    for di in range(KH):
        for dj in range(KW):
            bands[o + di, dj, o] = weight[di, dj]
    biasb = np.full((SLAB_IN, 1), bias[0], dtype=np.float32)

    starts = [min(i * ROWS_PER_CORE, H - SLAB_IN_ROWS) for i in range(N_CORES)]
    in_maps = [
        {
            "xs": np.ascontiguousarray(X[s0 : s0 + SLAB_IN_ROWS]),
            "bands": bands,
            "biasb": biasb,
        }
        for s0 in starts
    ]

    res = run_bass_kernel_spmd(nc, in_maps, core_ids=list(range(N_CORES)))
    LAST_RESULTS = res

    full = np.empty((OH, OW), dtype=np.float32)
    for i in range(N_CORES - 1):
        full[i * ROWS_PER_CORE : (i + 1) * ROWS_PER_CORE] = res.results[i]["out"]
    # Core 7's slab starts at row 7166, so its first 2 output rows duplicate
    # core 6's last 2; keep rows 2.. (= conv rows 7168..8189).
    full[(N_CORES - 1) * ROWS_PER_CORE :] = res.results[N_CORES - 1]["out"][
        ROWS_PER_CORE - (OH - (N_CORES - 1) * ROWS_PER_CORE) :
    ]
    return full


# revision 7
# speedup vs baseline: 1.1821x; 1.1821x over previous
"""Trainium2 Bass kernel: 2D valid cross-correlation (3x3) + bias on 8192x8192 fp32.

Strategy:
  - Row-shard X across 8 NeuronCores with a 2-row halo handled by host-side
    overlapped slicing (each core gets a 1026x8192 slab; core 7's slab is
    shifted up by 2 rows so all cores run an identical SPMD program).
  - On each core: conv2d(3x3) = 3 PSUM-accumulating matmuls per output tile
    with banded "shift" matrices built from the 3x3 weight. The banded matrix
    B_dj[p, o] = w[p - o, dj] turns the partition-axis (row) shifts into a
    matmul contraction; the column shifts dj are free-axis offsets of the rhs.
    float32r inputs run the PE at ~2 cycles/row (vs 4 for fp32) at ~tf32
    precision.
  - Loads/stores are split into ~1MB column chunks on separate HWDGE rings
    (loads on sync, stores on scalar) so compute starts ~5us in and the
    DMA engines stay saturated; weights/bias ride SWDGE (gpsimd).
  - Matmuls are grouped dj-outer over 4 PSUM banks (4 col-tiles share one
    LDWEIGHTS target) to keep PE bursts dense; DVE evacuates PSUM -> SBUF
    fused with the bias add.
"""

import os
import sys

import numpy as np

for _p in (
    "/opt/trn_rl_repo",
    "/root/.axon_site/_ro/trn_rl_repo",
    "/root/.axon_site/_ro/pypackages",
    "/opt/pypackages",
):
    if os.path.isdir(_p) and _p not in sys.path:
        sys.path.append(_p)

import concourse.bacc as bacc
import concourse.mybir as mybir
import concourse.tile as tile
from concourse.bass_utils import run_bass_kernel_spmd

N_CORES = 8
H = W = 8192
KH = KW = 3
OH = H - KH + 1  # 8190
OW = W - KW + 1  # 8190
ROWS_PER_CORE = 1024  # output rows produced per core (core 7: first 2 dropped)
SLAB_IN_ROWS = 1026  # input rows per core slab
SLAB_IN = 128  # input rows per row-slab tile
SLAB_OUT = 126  # output rows per row-slab tile
N_FULL_SLABS = 8  # 8 * 126 = 1008
TAIL_IN = SLAB_IN_ROWS - N_FULL_SLABS * SLAB_OUT  # 18
TAIL_OUT = ROWS_PER_CORE - N_FULL_SLABS * SLAB_OUT  # 16
COL_TILE = 512
N_COLT = (OW + COL_TILE - 1) // COL_TILE  # 16 (15*512 + 510)
GROUP = 4  # col-tiles per dj-outer matmul group (= PSUM banks per group)
N_GROUPS = N_COLT // GROUP  # 4

# Column chunking for DMA: ~1MB chunks so transfers stay efficient while the
# pipeline gets fine-grained dependencies (compute starts after chunk 0).
IN_CHUNKS = [(0, 2050), (2050, 4098), (4098, 6146), (6146, 8192)]
OUT_CHUNKS = [(0, 2048), (2048, 4096), (4096, 6144), (6144, 8190)]

_NC = None
LAST_RESULTS = None


def _build():
    nc = bacc.Bacc(
        "TRN2", target_bir_lowering=False, debug=False, num_devices=N_CORES
    )
    # float32r: same bits as fp32, but marks the data as feedable to the PE's
    # fp32r matmul mode (the BIR verifier requires fp32r matmul inputs to be
    # *produced* as fp32r, so the dtype is set at the DRAM/DMA level).
    xs = nc.dram_tensor(
        "xs", [SLAB_IN_ROWS, W], mybir.dt.float32r, kind="ExternalInput"
    )
    bands = nc.dram_tensor(
        "bands", [SLAB_IN, KW, SLAB_OUT], mybir.dt.float32r, kind="ExternalInput"
    )
    biasb = nc.dram_tensor("biasb", [SLAB_IN, 1], mybir.dt.float32, kind="ExternalInput")
    out = nc.dram_tensor(
        "out", [ROWS_PER_CORE, OW], mybir.dt.float32, kind="ExternalOutput"
    )

    f32r = mybir.dt.float32r
    f32 = mybir.dt.float32

    with tile.TileContext(nc) as tc:
        with (
            tc.tile_pool(name="const", bufs=1) as cpool,
            tc.tile_pool(name="inp", bufs=3) as ipool,
            tc.tile_pool(name="outp", bufs=2) as opool,
            tc.tile_pool(name="psum", bufs=2 * GROUP, space="PSUM") as pspool,
        ):
            # Consts ride the sync (HWDGE) ring ahead of the first slab chunks:
            # tiny transfers that gate the first matmul, so they go first.
            bt = cpool.tile([SLAB_IN, KW, SLAB_OUT], f32r, tag="bt")
            nc.sync.dma_start(bt[:], bands.ap())
            bias_t = cpool.tile([SLAB_IN, 1], f32, tag="bias")
            nc.sync.dma_start(bias_t[:], biasb.ap())

            for s in range(N_FULL_SLABS + 1):
                in_rows = SLAB_IN if s < N_FULL_SLABS else TAIL_IN
                out_rows = SLAB_OUT if s < N_FULL_SLABS else TAIL_OUT
                r0 = s * SLAB_OUT

                it = ipool.tile([SLAB_IN, W], f32r, tag="it", name=f"it{s}")
                for a, b in IN_CHUNKS:
                    nc.sync.dma_start(
                        it[:in_rows, a:b], xs.ap()[r0 : r0 + in_rows, a:b]
                    )

                ot = opool.tile([SLAB_OUT, OW], f32, tag="ot", name=f"ot{s}")

                for g in range(N_GROUPS):
                    pss = [
                        pspool.tile(
                            [SLAB_OUT, COL_TILE], f32, tag="ps", name=f"ps{s}_{g}_{t}"
                        )
                        for t in range(GROUP)
                    ]
                    for dj in range(KW):
                        for t in range(GROUP):
                            j = GROUP * g + t
                            c0 = j * COL_TILE
                            n = min(COL_TILE, OW - c0)
                            nc.tensor.matmul(
                                pss[t][:out_rows, :n],
                                bt[:in_rows, dj, :out_rows],
                                it[:in_rows, c0 + dj : c0 + dj + n],
                                start=(dj == 0),
                                stop=(dj == KW - 1),
                            )
                    for t in range(GROUP):
                        j = GROUP * g + t
                        c0 = j * COL_TILE
                        n = min(COL_TILE, OW - c0)
                        nc.vector.tensor_scalar_add(
                            ot[:out_rows, c0 : c0 + n],
                            pss[t][:out_rows, :n],
                            bias_t[:out_rows, :],
                        )
                    a, b = OUT_CHUNKS[g]
                    nc.scalar.dma_start(
                        out.ap()[r0 : r0 + out_rows, a:b], ot[:out_rows, a:b]
                    )

    nc.compile()
    return nc


def kernel(X, weight, bias):
    global _NC, LAST_RESULTS
    X = np.ascontiguousarray(np.asarray(X, dtype=np.float32))
    weight = np.asarray(weight, dtype=np.float32)
    bias = np.asarray(bias, dtype=np.float32).reshape(-1)

    if _NC is None:
        _NC = _build()
    nc = _NC

    # Banded shift matrices: bands[p, dj, o] = w[p - o, dj] for 0 <= p-o < 3.
    bands = np.zeros((SLAB_IN, KW, SLAB_OUT), dtype=np.float32)
    o = np.arange(SLAB_OUT)
    for di in range(KH):
        for dj in range(KW):
            bands[o + di, dj, o] = weight[di, dj]
    biasb = np.full((SLAB_IN, 1), bias[0], dtype=np.float32)

    starts = [min(i * ROWS_PER_CORE, H - SLAB_IN_ROWS) for i in range(N_CORES)]
    in_maps = [
        {
            "xs": np.ascontiguousarray(X[s0 : s0 + SLAB_IN_ROWS]),
            "bands": bands,
            "biasb": biasb,
        }
        for s0 in starts
    ]

    res = run_bass_kernel_spmd(nc, in_maps, core_ids=list(range(N_CORES)))
    LAST_RESULTS = res

    full = np.empty((OH, OW), dtype=np.float32)
    for i in range(N_CORES - 1):
        full[i * ROWS_PER_CORE : (i + 1) * ROWS_PER_CORE] = res.results[i]["out"]
    # Core 7's slab starts at row 7166, so its first 2 output rows duplicate
    # core 6's last 2; keep rows 2.. (= conv rows 7168..8189).
    full[(N_CORES - 1) * ROWS_PER_CORE :] = res.results[N_CORES - 1]["out"][
        ROWS_PER_CORE - (OH - (N_CORES - 1) * ROWS_PER_CORE) :
    ]
    return full
